# revision 1
# baseline (speedup 1.0000x reference)
"""GCN 3-layer kernel for Trainium2, 8-core SPMD.

Math (per layer, PyG GCN convention with self-loops, factorized):
    deg[d]  = indegree(d) + 1;  dinv = deg^-1/2
    y       = dinv[:,None] * (h @ W)                    (per-node scale)
    agg[d]  = sum_{e: dst[e]=d} y[src[e]]  + y[d]       (self-loop as edge)
    h_next  = dinv[:,None] * agg + b                    (+ relu on last layer)

Distribution: destination-sharded across 8 cores (6272 = 49*128 node slots
per core, padded to 50176 total).  Each core computes y for its own nodes,
an AllGather replicates the full y table (bf16) to every core's DRAM, then
each core gathers message rows with dma_gather and scatter-adds them with
one-hot matmuls on the PE (PSUM accumulation per 128-dst window).

dma_gather indices are int16, so the y table is addressed via two base
offsets (row 0 for src < 25088, row 17408 for src >= 25088; 50176-17408 =
32768 rows exactly covers the int16 range).
"""

import numpy as np
import ml_dtypes

N_NODES = 50000
N_CORES = 8
PER_CORE = 6272            # 49 * 128
N_PAD = PER_CORE * N_CORES # 50176
N_WIN = PER_CORE // 128    # 49
HI_BASE = 17408            # hi gather base row; 50176-17408 = 32768
LO_HI_SPLIT = 25088        # src < split -> lo stream, else hi
F = 128                    # feature width (layer3 padded 64->128)
F_OUT = 64
GROUP_WINDOWS = 5          # windows per gather chunk

BF16 = ml_dtypes.bfloat16


def _wrap_idx16(idx: np.ndarray) -> np.ndarray:
    """Wrap a flat int16 index stream into the [128, n/16] layout dma_gather
    expects (element i at [i%16, i//16], replicated across the 8 groups of
    16 partitions)."""
    n = len(idx)
    assert n % 128 == 0
    cols = n // 16
    out = np.empty((128, cols), np.int16)
    w = idx.reshape(cols, 16).T  # [16, cols]
    for g in range(8):
        out[g * 16:(g + 1) * 16, :] = w
    return out


def _preprocess(edge_index: np.ndarray):
    """Host-side graph prep: degree norm, dst-sharding, per-window edge
    streams (lo/hi by source row), block padding shared across cores."""
    src = edge_index[0].astype(np.int64)
    dst = edge_index[1].astype(np.int64)
    deg = np.bincount(dst, minlength=N_NODES).astype(np.float64) + 1.0
    dinv = (1.0 / np.sqrt(deg)).astype(np.float32)
    dinv_pad = np.ones(N_PAD, np.float32)
    dinv_pad[:N_NODES] = dinv

    # append self-edges
    selfn = np.arange(N_NODES, dtype=np.int64)
    src_a = np.concatenate([src, selfn])
    dst_a = np.concatenate([dst, selfn])

    core_of = dst_a // PER_CORE
    win_of = (dst_a % PER_CORE) // 128
    dloc_of = dst_a % 128
    is_lo = src_a < LO_HI_SPLIT

    # bucket edges: per (core, window, stream) lists of (idx16, dst_local)
    # sort once by (core, window)
    order = np.lexsort((dst_a, win_of, core_of))
    src_s, core_s, win_s, dloc_s, lo_s = (
        src_a[order], core_of[order], win_of[order], dloc_of[order], is_lo[order])

    # per (core, window, stream) counts
    counts = np.zeros((N_CORES, N_WIN, 2), np.int64)
    np.add.at(counts, (core_s, win_s, (~lo_s).astype(np.int64)), 1)
    # shared block counts per window (max over cores), at least 1 lo block
    blk_lo = np.maximum(1, -(-counts[:, :, 0].max(axis=0) // 128))  # [N_WIN]
    blk_hi = np.maximum(0, -(-counts[:, :, 1].max(axis=0) // 128))  # [N_WIN]

    # slot offsets within each stream
    off_lo = np.concatenate([[0], np.cumsum(blk_lo * 128)])
    off_hi = np.concatenate([[0], np.cumsum(blk_hi * 128)])
    n_lo, n_hi = int(off_lo[-1]), int(off_hi[-1])

    # fill per-core padded streams
    idx_lo = np.zeros((N_CORES, n_lo), np.int16)
    idx_hi = np.zeros((N_CORES, n_hi), np.int16)
    dl_lo = np.full((N_CORES, n_lo), 999.0, np.float32)
    dl_hi = np.full((N_CORES, n_hi), 999.0, np.float32)

    # boundaries of (core, window) groups in the sorted arrays
    keys = core_s * N_WIN + win_s
    bounds = np.searchsorted(keys, np.arange(N_CORES * N_WIN + 1))
    for c in range(N_CORES):
        for w in range(N_WIN):
            k = c * N_WIN + w
            sl = slice(bounds[k], bounds[k + 1])
            s_src = src_s[sl]; s_dl = dloc_s[sl]; s_lo = lo_s[sl]
            lo_src = s_src[s_lo]; lo_dl = s_dl[s_lo]
            hi_src = s_src[~s_lo]; hi_dl = s_dl[~s_lo]
            o = off_lo[w]
            idx_lo[c, o:o + len(lo_src)] = lo_src.astype(np.int16)
            dl_lo[c, o:o + len(lo_src)] = lo_dl
            o = off_hi[w]
            idx_hi[c, o:o + len(hi_src)] = (hi_src - HI_BASE).astype(np.int16)
            dl_hi[c, o:o + len(hi_src)] = hi_dl

    return dinv_pad, blk_lo, blk_hi, off_lo, off_hi, idx_lo, idx_hi, dl_lo, dl_hi


def _build_and_run(inputs_np, dinv_pad, blk_lo, blk_hi, off_lo, off_hi,
                   idx_lo, idx_hi, dl_lo, dl_hi, trace=False, sim=False):
    import concourse.bacc as bacc
    import concourse.mybir as mybir
    from concourse.tile import TileContext
    from concourse import bass, bass_utils, library_config
    from concourse.masks import make_identity

    x = inputs_np["x"]
    Ws = [np.asarray(inputs_np[k], np.float32) for k in ("W1", "W2", "W3")]
    bs = [np.asarray(inputs_np[k], np.float32) for k in ("b1", "b2", "b3")]
    # pad W3/b3 to 128 output features
    W3p = np.zeros((F, F), np.float32); W3p[:, :F_OUT] = Ws[2]
    b3p = np.zeros(F, np.float32); b3p[:F_OUT] = bs[2]
    Ws[2], bs[2] = W3p, b3p

    n_lo, n_hi = idx_lo.shape[1], idx_hi.shape[1]
    # gather groups of GROUP_WINDOWS windows
    groups = [list(range(g, min(g + GROUP_WINDOWS, N_WIN)))
              for g in range(0, N_WIN, GROUP_WINDOWS)]
    glo = [(int(off_lo[g[0]]), int(off_lo[g[-1] + 1])) for g in groups]
    ghi = [(int(off_hi[g[0]]), int(off_hi[g[-1] + 1])) for g in groups]
    cap_lo = max(b - a for a, b in glo) // 128
    cap_hi = max(1, max(b - a for a, b in ghi) // 128)

    nc = bacc.Bacc("TRN2", target_bir_lowering=False, debug=False, num_devices=N_CORES, num_swdge_queues=2)
    dt = mybir.dt

    # ---- kernel I/O -----------------------------------------------------
    t_xT = nc.dram_tensor("xT_own", [128, PER_CORE], dt.float32, kind="ExternalInput")
    t_W = [nc.dram_tensor(f"W{i+1}m", [F, F], dt.float32, kind="ExternalInput") for i in range(3)]
    t_b = [nc.dram_tensor(f"b{i+1}m", [128, F], dt.float32, kind="ExternalInput") for i in range(3)]
    t_dinv = nc.dram_tensor("dinv_own", [128, N_WIN], dt.float32, kind="ExternalInput")
    t_iota = nc.dram_tensor("iota", [128, 128], dt.bfloat16, kind="ExternalInput")
    t_ilo = nc.dram_tensor("idx_lo", [128, n_lo // 16], dt.int16, kind="ExternalInput")
    t_ihi = nc.dram_tensor("idx_hi", [128, max(1, n_hi // 16)], dt.int16, kind="ExternalInput")
    t_dlo = nc.dram_tensor("dl_lo", [128, n_lo // 128], dt.float32, kind="ExternalInput")
    t_dhi = nc.dram_tensor("dl_hi", [128, max(1, n_hi // 128)], dt.float32, kind="ExternalInput")
    t_out = nc.dram_tensor("h_out", [PER_CORE, F_OUT], dt.float32, kind="ExternalOutput")

    with TileContext(nc) as tc:
        nc.gpsimd.load_library(library_config.mlp)
        with tc.tile_pool(name="const", bufs=1) as cpool, \
             tc.tile_pool(name="state", bufs=1) as spool, \
             tc.tile_pool(name="gath", bufs=2) as gpool, \
             tc.tile_pool(name="work", bufs=3) as wpool, \
             tc.tile_pool(name="sbig", bufs=2) as sbig, \
             tc.tile_pool(name="psA", bufs=2, space="PSUM") as psA, \
             tc.tile_pool(name="psB", bufs=2, space="PSUM") as psB, \
             tc.tile_pool(name="psT", bufs=2, space="PSUM") as psT, \
             tc.tile_pool(name="dram", bufs=1, space="DRAM") as dpool:

            # ---- constants ----
            c_W = [cpool.tile([F, F], dt.float32, tag=f"W{i}", name=f"cW{i}") for i in range(3)]
            c_b = [cpool.tile([128, F], dt.float32, tag=f"b{i}", name=f"cb{i}") for i in range(3)]
            c_dinv = cpool.tile([128, N_WIN], dt.float32, tag="dinv", name="dinv")
            c_iota = cpool.tile([128, 128], dt.bfloat16, tag="iota", name="iota")
            c_ilo = cpool.tile([128, n_lo // 16], dt.int16, tag="ilo", name="ilo")
            c_ihi = cpool.tile([128, max(1, n_hi // 16)], dt.int16, tag="ihi", name="ihi")
            c_dlo = cpool.tile([128, n_lo // 128], dt.float32, tag="dlo", name="dlo")
            c_dhi = cpool.tile([128, max(1, n_hi // 128)], dt.float32, tag="dhi", name="dhi")
            c_ident = cpool.tile([128, 128], dt.float32, tag="ident", name="ident")
            for i in range(3):
                nc.sync.dma_start(c_W[i][:], t_W[i][:])
                nc.sync.dma_start(c_b[i][:], t_b[i][:])
            nc.sync.dma_start(c_dinv[:], t_dinv[:])
            nc.sync.dma_start(c_iota[:], t_iota[:])
            nc.sync.dma_start(c_ilo[:], t_ilo[:])
            nc.sync.dma_start(c_ihi[:], t_ihi[:])
            nc.sync.dma_start(c_dlo[:], t_dlo[:])
            nc.sync.dma_start(c_dhi[:], t_dhi[:])
            make_identity(nc, c_ident[:])

            # ---- persistent state ----
            hT = [spool.tile([128, PER_CORE], dt.float32, tag="hT_a", name="hT_a"),
                  spool.tile([128, PER_CORE], dt.float32, tag="hT_b", name="hT_b")]
            nc.sync.dma_start(hT[0][:], t_xT[:])
            y_sb = spool.tile([128, N_WIN, F], dt.bfloat16, tag="y_sb", name="y_sb")
            out_sb = spool.tile([128, N_WIN, F_OUT], dt.float32, tag="out_sb", name="out_sb")

            y_fulls = [dpool.tile([N_PAD, F], dt.bfloat16, addr_space="Shared",
                                  name=f"y_full{i}") for i in range(3)]
            ag_ins = [dpool.tile([PER_CORE, F], dt.bfloat16, name=f"ag_in{i}")
                      for i in range(3)]

            for layer in range(3):
                h_in = hT[layer % 2]
                h_out = hT[(layer + 1) % 2]
                # ---- phase A: y = dinv * (h @ W)  (own nodes) ----
                for t in range(N_WIN):
                    ps = psA.tile([128, F], dt.float32, tag="psA", space="PSUM")
                    nc.tensor.matmul(ps[:], lhsT=h_in[:, t * 128:(t + 1) * 128],
                                     rhs=c_W[layer][:], start=True, stop=True)
                    nc.vector.tensor_scalar(
                        out=y_sb[:, t, :], in0=ps[:],
                        scalar1=c_dinv[:, t:t + 1], scalar2=None,
                        op0=mybir.AluOpType.mult)
                ag_in = ag_ins[layer]
                y_full = y_fulls[layer]
                nc.sync.dma_start(
                    ag_in[:].rearrange("(t p) f -> p t f", p=128), y_sb[:])
                # ---- exchange: full y table ----
                nc.gpsimd.collective_compute(
                    "AllGather", mybir.AluOpType.bypass,
                    replica_groups=[list(range(N_CORES))],
                    ins=[ag_in.opt()], outs=[y_full.opt()])

                # ---- phase B: gather + one-hot matmul aggregation ----
                for gi, g in enumerate(groups):
                    lo_a, lo_b = glo[gi]
                    hi_a, hi_b = ghi[gi]
                    nlo = lo_b - lo_a
                    nhi = hi_b - hi_a
                    m_lo = gpool.tile([128, cap_lo, F], dt.bfloat16, tag="mlo", name="mlo")
                    m_hi = gpool.tile([128, cap_hi, F], dt.bfloat16, tag="mhi", name="mhi")
                    nc.gpsimd.dma_gather(
                        out_ap=m_lo[:, :nlo // 128, :], in_ap=y_full[:],
                        idxs_ap=c_ilo[:, lo_a // 16:lo_b // 16],
                        num_idxs=nlo, num_idxs_reg=nlo, elem_size=F,
                        queue_num=0, single_packet=False)
                    if nhi > 0:
                        nc.gpsimd.dma_gather(
                            out_ap=m_hi[:, :nhi // 128, :], in_ap=y_full[HI_BASE:, :],
                            idxs_ap=c_ihi[:, hi_a // 16:hi_b // 16],
                            num_idxs=nhi, num_idxs_reg=nhi, elem_size=F,
                            queue_num=1, single_packet=False)
                    for w in g:
                        nblk = int(blk_lo[w] + blk_hi[w])
                        agg = psB.tile([128, F], dt.float32, tag="agg", space="PSUM")
                        k = 0
                        for b in range(int(blk_lo[w])):
                            B = int(off_lo[w]) // 128 + b
                            S = wpool.tile([128, 128], dt.bfloat16, tag="S", name="S")
                            nc.vector.tensor_scalar(
                                out=S[:], in0=c_iota[:],
                                scalar1=c_dlo[:, B:B + 1], scalar2=None,
                                op0=mybir.AluOpType.is_equal)
                            nc.tensor.matmul(
                                agg[:], lhsT=S[:],
                                rhs=m_lo[:, B - lo_a // 128, :],
                                start=(k == 0), stop=(k == nblk - 1))
                            k += 1
                        for b in range(int(blk_hi[w])):
                            B = int(off_hi[w]) // 128 + b
                            S = wpool.tile([128, 128], dt.bfloat16, tag="S", name="S")
                            nc.vector.tensor_scalar(
                                out=S[:], in0=c_iota[:],
                                scalar1=c_dhi[:, B:B + 1], scalar2=None,
                                op0=mybir.AluOpType.is_equal)
                            nc.tensor.matmul(
                                agg[:], lhsT=S[:],
                                rhs=m_hi[:, B - hi_a // 128, :],
                                start=(k == 0), stop=(k == nblk - 1))
                            k += 1
                        # ---- epilogue: h = dinv*agg + b ----
                        hs = wpool.tile([128, F], dt.float32, tag="hs", name="hs")
                        nc.vector.tensor_scalar(
                            out=hs[:], in0=agg[:],
                            scalar1=c_dinv[:, w:w + 1], scalar2=None,
                            op0=mybir.AluOpType.mult)
                        if layer < 2:
                            hb = wpool.tile([128, F], dt.float32, tag="hb", name="hb")
                            nc.vector.tensor_add(hb[:], hs[:], c_b[layer][:])
                            tp = psT.tile([128, 128], dt.float32, tag="tp", space="PSUM")
                            nc.tensor.transpose(tp[:], hb[:], c_ident[:])
                            nc.vector.tensor_copy(
                                out=h_out[:, w * 128:(w + 1) * 128], in_=tp[:])
                        else:
                            hb = wpool.tile([128, F], dt.float32, tag="hb", name="hb")
                            nc.vector.tensor_add(hb[:], hs[:], c_b[layer][:])
                            nc.vector.tensor_scalar(
                                out=out_sb[:, w, :], in0=hb[:, :F_OUT],
                                scalar1=0.0, scalar2=None,
                                op0=mybir.AluOpType.max)
            nc.sync.dma_start(
                t_out[:].rearrange("(t p) f -> p t f", p=128), out_sb[:])

    nc.compile()

    # ---- per-core inputs ----
    xT_all = np.zeros((128, N_PAD), np.float32)
    xT_all[:, :N_NODES] = np.asarray(x, np.float32).T
    iota_m = np.broadcast_to(np.arange(128, dtype=np.float32), (128, 128)).astype(BF16)
    in_maps = []
    for c in range(N_CORES):
        rows = slice(c * PER_CORE, (c + 1) * PER_CORE)
        din = dinv_pad[rows].reshape(N_WIN, 128).T.copy()  # [128, N_WIN]
        in_map = {
            "xT_own": np.ascontiguousarray(xT_all[:, rows]),
            "dinv_own": din,
            "iota": iota_m.copy(),
            "idx_lo": _wrap_idx16(idx_lo[c]),
            "idx_hi": _wrap_idx16(idx_hi[c]) if n_hi else np.zeros((128, 1), np.int16),
            "dl_lo": dl_lo[c].reshape(-1, 128).T.copy(),
            "dl_hi": (dl_hi[c].reshape(-1, 128).T.copy() if n_hi
                      else np.zeros((128, 1), np.float32)),
        }
        for i in range(3):
            in_map[f"W{i+1}m"] = Ws[i].copy()
            in_map[f"b{i+1}m"] = np.broadcast_to(bs[i], (128, F)).copy()
        in_maps.append(in_map)

    if sim:
        from concourse.bass_interp import MultiCoreSim
        mcs = MultiCoreSim(nc, num_cores=N_CORES, trace=False,
                           require_finite=False, require_nnan=False)
        for ci, core in enumerate(mcs.cores.values()):
            for k, v in in_maps[ci].items():
                core.tensor(k)[:] = v
        mcs.simulate(check_with_hw=False)
        outs = [np.asarray(core.tensor("h_out"))
                for core in mcs.cores.values()]
        res = None
    else:
        res = bass_utils.run_bass_kernel_spmd(
            nc, in_maps, core_ids=list(range(N_CORES)), trace=trace)
        outs = [r["h_out"] for r in res.results]
    full = np.concatenate(outs, axis=0)[:N_NODES]
    return full, res


def kernel(**inputs) -> np.ndarray:
    edge_index = np.asarray(inputs["edge_index"])
    prep = _preprocess(edge_index)
    out, _ = _build_and_run(inputs, *prep)
    return out



# revision 5
# speedup vs baseline: 1.3371x; 1.3371x over previous
"""GCN 3-layer kernel for Trainium2, 8-core SPMD.

Math (per layer, PyG GCN convention with self-loops, factorized):
    deg[d]  = indegree(d) + 1;  dinv = deg^-1/2
    y       = dinv[:,None] * (h @ W)                    (per-node scale)
    agg[d]  = sum_{e: dst[e]=d} y[src[e]]  + y[d]       (self-loop as edge)
    h_next  = dinv[:,None] * agg + b                    (+ relu on last layer)

Distribution: destination-sharded across 8 cores (6272 = 49*128 node slots
per core, padded to 50176 total).  Each core computes y for its own nodes,
an AllGather replicates the full y table (bf16) to every core's DRAM, then
each core gathers message rows with dma_gather and scatter-adds them with
one-hot matmuls on the PE (PSUM accumulation per 128-dst window).

Scatter one-hot matrices S[e, d] = (dl[e] == d) are built in batches on the
DVE: one tensor_tensor(is_equal) per (window, stream) in a [128e, 128d, nb]
layout (d on the middle dim, block on the packed last dim) so the op runs in
the 2x 16-bit DVE mode; block j is consumed by the PE as the strided slice
S[:, :, j].

Gathers are issued as prepare_only + trigger_dma so the Pool engine is not
blocked while the DMA rings drain; Tile attributes the m_lo/m_hi writes to
the DMA-completion semaphore, so consumers sync correctly.

dma_gather indices are int16, so the y table is addressed via two base
offsets (row 0 for src < 25088, row 17408 for src >= 25088; 50176-17408 =
32768 rows exactly covers the int16 range).
"""

import numpy as np
import ml_dtypes

N_NODES = 50000
N_CORES = 8
PER_CORE = 6272            # 49 * 128
N_PAD = PER_CORE * N_CORES # 50176
N_WIN = PER_CORE // 128    # 49
HI_BASE = 17408            # hi gather base row; 50176-17408 = 32768
LO_HI_SPLIT = 25088        # src < split -> lo stream, else hi
F = 128                    # feature width (layer3 padded 64->128)
F_OUT = 64
GROUP_WINDOWS = 5          # windows per gather chunk

BF16 = ml_dtypes.bfloat16


def _wrap_idx16(idx: np.ndarray) -> np.ndarray:
    """Wrap a flat int16 index stream into the [128, n/16] layout dma_gather
    expects (element i at [i%16, i//16], replicated across the 8 groups of
    16 partitions)."""
    n = len(idx)
    assert n % 128 == 0
    cols = n // 16
    out = np.empty((128, cols), np.int16)
    w = idx.reshape(cols, 16).T  # [16, cols]
    for g in range(8):
        out[g * 16:(g + 1) * 16, :] = w
    return out


def _preprocess(edge_index: np.ndarray):
    """Host-side graph prep: degree norm, dst-sharding, per-window edge
    streams (lo/hi by source row), block padding shared across cores."""
    src = edge_index[0].astype(np.int64)
    dst = edge_index[1].astype(np.int64)
    deg = np.bincount(dst, minlength=N_NODES).astype(np.float64) + 1.0
    dinv = (1.0 / np.sqrt(deg)).astype(np.float32)
    dinv_pad = np.ones(N_PAD, np.float32)
    dinv_pad[:N_NODES] = dinv

    # append self-edges
    selfn = np.arange(N_NODES, dtype=np.int64)
    src_a = np.concatenate([src, selfn])
    dst_a = np.concatenate([dst, selfn])

    core_of = dst_a // PER_CORE
    win_of = (dst_a % PER_CORE) // 128
    dloc_of = dst_a % 128
    is_lo = src_a < LO_HI_SPLIT

    # bucket edges: per (core, window, stream) lists of (idx16, dst_local)
    # sort once by (core, window)
    order = np.lexsort((dst_a, win_of, core_of))
    src_s, core_s, win_s, dloc_s, lo_s = (
        src_a[order], core_of[order], win_of[order], dloc_of[order], is_lo[order])

    # per (core, window, stream) counts
    counts = np.zeros((N_CORES, N_WIN, 2), np.int64)
    np.add.at(counts, (core_s, win_s, (~lo_s).astype(np.int64)), 1)
    # shared block counts per window (max over cores), at least 1 lo block
    blk_lo = np.maximum(1, -(-counts[:, :, 0].max(axis=0) // 128))  # [N_WIN]
    blk_hi = np.maximum(0, -(-counts[:, :, 1].max(axis=0) // 128))  # [N_WIN]

    # slot offsets within each stream
    off_lo = np.concatenate([[0], np.cumsum(blk_lo * 128)])
    off_hi = np.concatenate([[0], np.cumsum(blk_hi * 128)])
    n_lo, n_hi = int(off_lo[-1]), int(off_hi[-1])

    # fill per-core padded streams
    idx_lo = np.zeros((N_CORES, n_lo), np.int16)
    idx_hi = np.zeros((N_CORES, n_hi), np.int16)
    dl_lo = np.full((N_CORES, n_lo), 999.0, np.float32)
    dl_hi = np.full((N_CORES, n_hi), 999.0, np.float32)

    # boundaries of (core, window) groups in the sorted arrays
    keys = core_s * N_WIN + win_s
    bounds = np.searchsorted(keys, np.arange(N_CORES * N_WIN + 1))
    for c in range(N_CORES):
        for w in range(N_WIN):
            k = c * N_WIN + w
            sl = slice(bounds[k], bounds[k + 1])
            s_src = src_s[sl]; s_dl = dloc_s[sl]; s_lo = lo_s[sl]
            lo_src = s_src[s_lo]; lo_dl = s_dl[s_lo]
            hi_src = s_src[~s_lo]; hi_dl = s_dl[~s_lo]
            # sort each stream by source row: ascending DRAM addresses give
            # the memory controller page locality during the gather
            o_lo = np.argsort(lo_src, kind="stable")
            lo_src, lo_dl = lo_src[o_lo], lo_dl[o_lo]
            o_hi = np.argsort(hi_src, kind="stable")
            hi_src, hi_dl = hi_src[o_hi], hi_dl[o_hi]
            o = off_lo[w]
            idx_lo[c, o:o + len(lo_src)] = lo_src.astype(np.int16)
            dl_lo[c, o:o + len(lo_src)] = lo_dl
            o = off_hi[w]
            idx_hi[c, o:o + len(hi_src)] = (hi_src - HI_BASE).astype(np.int16)
            dl_hi[c, o:o + len(hi_src)] = hi_dl

    return dinv_pad, blk_lo, blk_hi, off_lo, off_hi, idx_lo, idx_hi, dl_lo, dl_hi


def _build_and_run(inputs_np, dinv_pad, blk_lo, blk_hi, off_lo, off_hi,
                   idx_lo, idx_hi, dl_lo, dl_hi, trace=False, sim=False):
    import concourse.bacc as bacc
    import concourse.mybir as mybir
    from concourse.tile import TileContext
    from concourse import bass, bass_utils, library_config
    from concourse.masks import make_identity

    x = inputs_np["x"]
    Ws = [np.asarray(inputs_np[k], np.float32) for k in ("W1", "W2", "W3")]
    bs = [np.asarray(inputs_np[k], np.float32) for k in ("b1", "b2", "b3")]
    # pad W3/b3 to 128 output features
    W3p = np.zeros((F, F), np.float32); W3p[:, :F_OUT] = Ws[2]
    b3p = np.zeros(F, np.float32); b3p[:F_OUT] = bs[2]
    Ws[2], bs[2] = W3p, b3p

    n_lo, n_hi = idx_lo.shape[1], idx_hi.shape[1]
    NBMAX = int(max(blk_lo.max(), blk_hi.max() if n_hi else 1))
    # gather groups of GROUP_WINDOWS windows
    groups = [list(range(g, min(g + GROUP_WINDOWS, N_WIN)))
              for g in range(0, N_WIN, GROUP_WINDOWS)]
    glo = [(int(off_lo[g[0]]), int(off_lo[g[-1] + 1])) for g in groups]
    ghi = [(int(off_hi[g[0]]), int(off_hi[g[-1] + 1])) for g in groups]
    cap_lo = max(b - a for a, b in glo) // 128
    cap_hi = max(1, max(b - a for a, b in ghi) // 128)

    nc = bacc.Bacc("TRN2", target_bir_lowering=False, debug=False, num_devices=N_CORES, num_swdge_queues=2)
    dt = mybir.dt

    # ---- kernel I/O -----------------------------------------------------
    t_xT = nc.dram_tensor("xT_own", [128, PER_CORE], dt.float32, kind="ExternalInput")
    t_W = [nc.dram_tensor(f"W{i+1}m", [F, F], dt.float32, kind="ExternalInput") for i in range(3)]
    t_b = [nc.dram_tensor(f"b{i+1}m", [128, F], dt.float32, kind="ExternalInput") for i in range(3)]
    t_dinv = nc.dram_tensor("dinv_own", [128, N_WIN], dt.float32, kind="ExternalInput")
    t_iota3 = nc.dram_tensor("iota3", [128, 128, NBMAX], dt.bfloat16, kind="ExternalInput")
    t_ilo = nc.dram_tensor("idx_lo", [128, n_lo // 16], dt.int16, kind="ExternalInput")
    t_ihi = nc.dram_tensor("idx_hi", [128, max(1, n_hi // 16)], dt.int16, kind="ExternalInput")
    t_dlo = nc.dram_tensor("dl_lo", [128, n_lo // 128], dt.bfloat16, kind="ExternalInput")
    t_dhi = nc.dram_tensor("dl_hi", [128, max(1, n_hi // 128)], dt.bfloat16, kind="ExternalInput")
    t_out = nc.dram_tensor("h_out", [PER_CORE, F_OUT], dt.float32, kind="ExternalOutput")

    with TileContext(nc) as tc:
        nc.gpsimd.load_library(library_config.mlp)
        with tc.tile_pool(name="const", bufs=1) as cpool, \
             tc.tile_pool(name="state", bufs=1) as spool, \
             tc.tile_pool(name="gath", bufs=2) as gpool, \
             tc.tile_pool(name="sbld", bufs=2) as sbld, \
             tc.tile_pool(name="work", bufs=3) as wpool, \
             tc.tile_pool(name="psA", bufs=2, space="PSUM") as psA, \
             tc.tile_pool(name="psB", bufs=2, space="PSUM") as psB, \
             tc.tile_pool(name="psT", bufs=2, space="PSUM") as psT, \
             tc.tile_pool(name="dram", bufs=1, space="DRAM") as dpool:

            # ---- constants ----
            c_W = [cpool.tile([F, F], dt.float32, tag=f"W{i}", name=f"cW{i}") for i in range(3)]
            c_b = [cpool.tile([128, F], dt.float32, tag=f"b{i}", name=f"cb{i}") for i in range(3)]
            c_dinv = cpool.tile([128, N_WIN], dt.float32, tag="dinv", name="dinv")
            c_iota3 = cpool.tile([128, 128, NBMAX], dt.bfloat16, tag="iota3", name="iota3")
            c_ilo = cpool.tile([128, n_lo // 16], dt.int16, tag="ilo", name="ilo")
            c_ihi = cpool.tile([128, max(1, n_hi // 16)], dt.int16, tag="ihi", name="ihi")
            c_dlo = cpool.tile([128, n_lo // 128], dt.bfloat16, tag="dlo", name="dlo")
            c_dhi = cpool.tile([128, max(1, n_hi // 128)], dt.bfloat16, tag="dhi", name="dhi")
            c_ident = cpool.tile([128, 128], dt.float32, tag="ident", name="ident")
            for i in range(3):
                nc.sync.dma_start(c_W[i][:], t_W[i][:])
                nc.sync.dma_start(c_b[i][:], t_b[i][:])
            nc.sync.dma_start(c_dinv[:], t_dinv[:])
            nc.sync.dma_start(c_iota3[:], t_iota3[:])
            nc.sync.dma_start(c_ilo[:], t_ilo[:])
            nc.sync.dma_start(c_ihi[:], t_ihi[:])
            nc.sync.dma_start(c_dlo[:], t_dlo[:])
            nc.sync.dma_start(c_dhi[:], t_dhi[:])
            make_identity(nc, c_ident[:])

            # ---- persistent state ----
            hT = [spool.tile([128, PER_CORE], dt.float32, tag="hT_a", name="hT_a"),
                  spool.tile([128, PER_CORE], dt.float32, tag="hT_b", name="hT_b")]
            nc.sync.dma_start(hT[0][:], t_xT[:])
            y_sb = spool.tile([128, N_WIN, F], dt.bfloat16, tag="y_sb", name="y_sb")
            out_sb = spool.tile([128, N_WIN, F_OUT], dt.float32, tag="out_sb", name="out_sb")

            y_fulls = [dpool.tile([N_PAD, F], dt.bfloat16, addr_space="Shared",
                                  name=f"y_full{i}") for i in range(3)]
            ag_ins = [dpool.tile([PER_CORE, F], dt.bfloat16, name=f"ag_in{i}")
                      for i in range(3)]

            def build_S(pool_tag, dl_tile, B0, nb):
                """One-hot scatter blocks for a (window, stream):
                S[e, d, j] = (dl[e, B0+j] == d), bf16, 2x DVE mode."""
                S = sbld.tile([128, 128, NBMAX], dt.bfloat16, tag=pool_tag,
                              name=pool_tag)
                dl_b = dl_tile[:, B0:B0 + nb].unsqueeze(1).broadcast_to(
                    [128, 128, nb])
                nc.vector.tensor_tensor(
                    out=S[:, :, :nb], in0=dl_b, in1=c_iota3[:, :, :nb],
                    op=mybir.AluOpType.is_equal)
                return S

            for layer in range(3):
                h_in = hT[layer % 2]
                h_out = hT[(layer + 1) % 2]
                # ---- phase A: y = dinv * (h @ W)  (own nodes) ----
                for t in range(N_WIN):
                    ps = psA.tile([128, F], dt.float32, tag="psA", space="PSUM")
                    nc.tensor.matmul(ps[:], lhsT=h_in[:, t * 128:(t + 1) * 128],
                                     rhs=c_W[layer][:], start=True, stop=True)
                    nc.vector.tensor_scalar(
                        out=y_sb[:, t, :], in0=ps[:],
                        scalar1=c_dinv[:, t:t + 1], scalar2=None,
                        op0=mybir.AluOpType.mult)
                ag_in = ag_ins[layer]
                y_full = y_fulls[layer]
                nc.sync.dma_start(
                    ag_in[:].rearrange("(t p) f -> p t f", p=128), y_sb[:])
                # ---- exchange: full y table ----
                nc.gpsimd.collective_compute(
                    "AllGather", mybir.AluOpType.bypass,
                    replica_groups=[list(range(N_CORES))],
                    ins=[ag_in.opt()], outs=[y_full.opt()])

                # ---- phase B: gather + one-hot matmul aggregation ----
                for gi, g in enumerate(groups):
                    lo_a, lo_b = glo[gi]
                    hi_a, hi_b = ghi[gi]
                    nlo = lo_b - lo_a
                    nhi = hi_b - hi_a
                    m_lo = gpool.tile([128, cap_lo, F], dt.bfloat16, tag="mlo", name="mlo")
                    m_hi = gpool.tile([128, cap_hi, F], dt.bfloat16, tag="mhi", name="mhi")
                    nc.gpsimd.dma_gather(
                        out_ap=m_lo[:, :nlo // 128, :], in_ap=y_full[:],
                        idxs_ap=c_ilo[:, lo_a // 16:lo_b // 16],
                        num_idxs=nlo, num_idxs_reg=nlo, elem_size=F,
                        queue_num=0, single_packet=False)
                    if nhi > 0:
                        nc.gpsimd.dma_gather(
                            out_ap=m_hi[:, :nhi // 128, :], in_ap=y_full[HI_BASE:, :],
                            idxs_ap=c_ihi[:, hi_a // 16:hi_b // 16],
                            num_idxs=nhi, num_idxs_reg=nhi, elem_size=F,
                            queue_num=1, single_packet=False)
                    for w in g:
                        nb_lo = int(blk_lo[w])
                        nb_hi = int(blk_hi[w])
                        nblk = nb_lo + nb_hi
                        B_lo = int(off_lo[w]) // 128
                        B_hi = int(off_hi[w]) // 128
                        S_lo = build_S("Slo", c_dlo, B_lo, nb_lo)
                        S_hi = build_S("Shi", c_dhi, B_hi, nb_hi) if nb_hi else None
                        agg = psB.tile([128, F], dt.float32, tag="agg", space="PSUM")
                        k = 0
                        for j in range(nb_lo):
                            nc.tensor.matmul(
                                agg[:], lhsT=S_lo[:, :, j],
                                rhs=m_lo[:, B_lo - lo_a // 128 + j, :],
                                start=(k == 0), stop=(k == nblk - 1))
                            k += 1
                        for j in range(nb_hi):
                            nc.tensor.matmul(
                                agg[:], lhsT=S_hi[:, :, j],
                                rhs=m_hi[:, B_hi - hi_a // 128 + j, :],
                                start=(k == 0), stop=(k == nblk - 1))
                            k += 1
                        # ---- epilogue: h = dinv*agg + b ----
                        hb = wpool.tile([128, F], dt.float32, tag="hb", name="hb")
                        nc.vector.scalar_tensor_tensor(
                            out=hb[:], in0=agg[:],
                            scalar=c_dinv[:, w:w + 1], in1=c_b[layer][:],
                            op0=mybir.AluOpType.mult, op1=mybir.AluOpType.add)
                        if layer < 2:
                            tp = psT.tile([128, 128], dt.float32, tag="tp", space="PSUM")
                            nc.tensor.transpose(tp[:], hb[:], c_ident[:])
                            nc.vector.tensor_copy(
                                out=h_out[:, w * 128:(w + 1) * 128], in_=tp[:])
                        else:
                            nc.vector.tensor_scalar(
                                out=out_sb[:, w, :], in0=hb[:, :F_OUT],
                                scalar1=0.0, scalar2=None,
                                op0=mybir.AluOpType.max)
            nc.sync.dma_start(
                t_out[:].rearrange("(t p) f -> p t f", p=128), out_sb[:])

    nc.compile()

    # ---- per-core inputs ----
    xT_all = np.zeros((128, N_PAD), np.float32)
    xT_all[:, :N_NODES] = np.asarray(x, np.float32).T
    iota3 = np.broadcast_to(
        np.arange(128, dtype=np.float32)[None, :, None],
        (128, 128, NBMAX)).astype(BF16)
    in_maps = []
    for c in range(N_CORES):
        rows = slice(c * PER_CORE, (c + 1) * PER_CORE)
        din = dinv_pad[rows].reshape(N_WIN, 128).T.copy()  # [128, N_WIN]
        in_map = {
            "xT_own": np.ascontiguousarray(xT_all[:, rows]),
            "dinv_own": din,
            "iota3": iota3.copy(),
            "idx_lo": _wrap_idx16(idx_lo[c]),
            "idx_hi": _wrap_idx16(idx_hi[c]) if n_hi else np.zeros((128, 1), np.int16),
            "dl_lo": dl_lo[c].reshape(-1, 128).T.astype(BF16).copy(),
            "dl_hi": (dl_hi[c].reshape(-1, 128).T.astype(BF16).copy() if n_hi
                      else np.zeros((128, 1), BF16)),
        }
        for i in range(3):
            in_map[f"W{i+1}m"] = Ws[i].copy()
            in_map[f"b{i+1}m"] = np.broadcast_to(bs[i], (128, F)).copy()
        in_maps.append(in_map)

    if sim:
        from concourse.bass_interp import MultiCoreSim
        mcs = MultiCoreSim(nc, num_cores=N_CORES, trace=False,
                           require_finite=False, require_nnan=False)
        for ci, core in enumerate(mcs.cores.values()):
            for k, v in in_maps[ci].items():
                core.tensor(k)[:] = v
        mcs.simulate(check_with_hw=False)
        outs = [np.asarray(core.tensor("h_out"))
                for core in mcs.cores.values()]
        res = None
    else:
        res = bass_utils.run_bass_kernel_spmd(
            nc, in_maps, core_ids=list(range(N_CORES)), trace=trace)
        outs = [r["h_out"] for r in res.results]
    full = np.concatenate(outs, axis=0)[:N_NODES]
    return full, res


def kernel(**inputs) -> np.ndarray:
    edge_index = np.asarray(inputs["edge_index"])
    prep = _preprocess(edge_index)
    out, _ = _build_and_run(inputs, *prep)
    return out


# revision 6
# speedup vs baseline: 1.8214x; 1.3623x over previous
"""GCN 3-layer kernel for Trainium2, 8-core SPMD.

Math (per layer, PyG GCN convention with self-loops, factorized):
    deg[d]  = indegree(d) + 1;  dinv = deg^-1/2
    y       = dinv[:,None] * (h @ W)                    (per-node scale)
    agg[d]  = sum_{e: dst[e]=d} y[src[e]]  + y[d]       (self-loop as edge)
    h_next  = dinv[:,None] * agg + b                    (+ relu on last layer)

Distribution: destination-sharded across 8 cores (6272 = 49*128 node slots
per core, padded to 50176 total).  The per-layer y table is exchanged in
TWO AllGather halves split by window (windows 0-24 -> table A of 25600
rows, windows 25-48 -> table B of 24576 rows) so the first half of the
exchange overlaps the tail of the previous layer's aggregation: y for
layer L+1 is computed inside layer L's per-window epilogue (phase-A fold),
and AG_A fires as soon as window 24's y is out.

Aggregation: per 128-dst window, gather message rows with dma_gather
(4 SWDGE queues; stream A split over q0/q1, stream B over q2/q3; indices
sorted by source row for DRAM page locality) and scatter-add them with
one-hot matmuls on the PE (PSUM accumulation per window).  One-hot blocks
S[e, d, j] = (dl[e, j] == d) are built in batches on the DVE: one
tensor_tensor(is_equal) per (window, stream) with a stride-0 broadcast of
dl against a materialized iota, consumed by the PE as strided slices
S[:, :, j].

Gather indices are int16: both tables are < 32768 rows, so no base-offset
tricks are needed.
"""

import numpy as np
import ml_dtypes

N_NODES = 50000
N_CORES = 8
PER_CORE = 6272            # 49 * 128
N_PAD = PER_CORE * N_CORES # 50176
N_WIN = PER_CORE // 128    # 49
A_WINS = 25                # windows 0..24 -> table A
B_WINS = N_WIN - A_WINS    # windows 25..48 -> table B
A_PC = A_WINS * 128        # 3200 rows per core in A
B_PC = B_WINS * 128        # 3072 rows per core in B
A_ROWS = A_PC * N_CORES    # 25600
B_ROWS = B_PC * N_CORES    # 24576
F = 128                    # feature width (layer3 padded 64->128)
F_OUT = 64
GROUP_WINDOWS = 5          # windows per gather chunk

BF16 = ml_dtypes.bfloat16


def _wrap_idx16(idx: np.ndarray) -> np.ndarray:
    """Wrap a flat int16 index stream into the [128, n/16] layout dma_gather
    expects (element i at [i%16, i//16], replicated across the 8 groups of
    16 partitions)."""
    n = len(idx)
    assert n % 128 == 0
    cols = n // 16
    out = np.empty((128, cols), np.int16)
    w = idx.reshape(cols, 16).T  # [16, cols]
    for g in range(8):
        out[g * 16:(g + 1) * 16, :] = w
    return out


def _preprocess(edge_index: np.ndarray):
    """Host-side graph prep: degree norm, dst-sharding, per-window edge
    streams (A/B by source window), block padding shared across cores."""
    src = edge_index[0].astype(np.int64)
    dst = edge_index[1].astype(np.int64)
    deg = np.bincount(dst, minlength=N_NODES).astype(np.float64) + 1.0
    dinv = (1.0 / np.sqrt(deg)).astype(np.float32)
    dinv_pad = np.ones(N_PAD, np.float32)
    dinv_pad[:N_NODES] = dinv

    # append self-edges
    selfn = np.arange(N_NODES, dtype=np.int64)
    src_a = np.concatenate([src, selfn])
    dst_a = np.concatenate([dst, selfn])

    core_of = dst_a // PER_CORE
    win_of = (dst_a % PER_CORE) // 128
    dloc_of = dst_a % 128
    src_core = src_a // PER_CORE
    src_loc = src_a % PER_CORE
    is_lo = src_loc < A_PC          # stream A: source window < 25
    # relabeled gather rows in each half-table
    gidx_a = src_core * A_PC + src_loc
    gidx_b = src_core * B_PC + (src_loc - A_PC)

    # sort once by (core, window)
    order = np.lexsort((dst_a, win_of, core_of))
    core_s, win_s, dloc_s, lo_s = (
        core_of[order], win_of[order], dloc_of[order], is_lo[order])
    ga_s, gb_s = gidx_a[order], gidx_b[order]

    # per (core, window, stream) counts
    counts = np.zeros((N_CORES, N_WIN, 2), np.int64)
    np.add.at(counts, (core_s, win_s, (~lo_s).astype(np.int64)), 1)
    # shared block counts per window (max over cores), at least 1 block
    blk_lo = np.maximum(1, -(-counts[:, :, 0].max(axis=0) // 128))  # [N_WIN]
    blk_hi = np.maximum(1, -(-counts[:, :, 1].max(axis=0) // 128))  # [N_WIN]

    # slot offsets within each stream
    off_lo = np.concatenate([[0], np.cumsum(blk_lo * 128)])
    off_hi = np.concatenate([[0], np.cumsum(blk_hi * 128)])
    n_lo, n_hi = int(off_lo[-1]), int(off_hi[-1])

    # fill per-core padded streams
    idx_lo = np.zeros((N_CORES, n_lo), np.int16)
    idx_hi = np.zeros((N_CORES, n_hi), np.int16)
    dl_lo = np.full((N_CORES, n_lo), 999.0, np.float32)
    dl_hi = np.full((N_CORES, n_hi), 999.0, np.float32)

    # boundaries of (core, window) groups in the sorted arrays
    keys = core_s * N_WIN + win_s
    bounds = np.searchsorted(keys, np.arange(N_CORES * N_WIN + 1))
    for c in range(N_CORES):
        for w in range(N_WIN):
            k = c * N_WIN + w
            sl = slice(bounds[k], bounds[k + 1])
            s_dl = dloc_s[sl]; s_lo = lo_s[sl]
            lo_src = ga_s[sl][s_lo]; lo_dl = s_dl[s_lo]
            hi_src = gb_s[sl][~s_lo]; hi_dl = s_dl[~s_lo]
            # sort each stream by source row: ascending DRAM addresses give
            # the memory controller page locality during the gather
            o_lo = np.argsort(lo_src, kind="stable")
            lo_src, lo_dl = lo_src[o_lo], lo_dl[o_lo]
            o_hi = np.argsort(hi_src, kind="stable")
            hi_src, hi_dl = hi_src[o_hi], hi_dl[o_hi]
            o = off_lo[w]
            idx_lo[c, o:o + len(lo_src)] = lo_src.astype(np.int16)
            dl_lo[c, o:o + len(lo_src)] = lo_dl
            o = off_hi[w]
            idx_hi[c, o:o + len(hi_src)] = hi_src.astype(np.int16)
            dl_hi[c, o:o + len(hi_src)] = hi_dl

    return dinv_pad, blk_lo, blk_hi, off_lo, off_hi, idx_lo, idx_hi, dl_lo, dl_hi


def _build_and_run(inputs_np, dinv_pad, blk_lo, blk_hi, off_lo, off_hi,
                   idx_lo, idx_hi, dl_lo, dl_hi, trace=False, sim=False):
    import concourse.bacc as bacc
    import concourse.mybir as mybir
    from concourse.tile import TileContext
    from concourse import bass, bass_utils, library_config
    from concourse.masks import make_identity

    x = inputs_np["x"]
    Ws = [np.asarray(inputs_np[k], np.float32) for k in ("W1", "W2", "W3")]
    bs = [np.asarray(inputs_np[k], np.float32) for k in ("b1", "b2", "b3")]
    # pad W3/b3 to 128 output features
    W3p = np.zeros((F, F), np.float32); W3p[:, :F_OUT] = Ws[2]
    b3p = np.zeros(F, np.float32); b3p[:F_OUT] = bs[2]
    Ws[2], bs[2] = W3p, b3p

    n_lo, n_hi = idx_lo.shape[1], idx_hi.shape[1]
    NBMAX = int(max(blk_lo.max(), blk_hi.max()))
    # gather groups of GROUP_WINDOWS windows
    groups = [list(range(g, min(g + GROUP_WINDOWS, N_WIN)))
              for g in range(0, N_WIN, GROUP_WINDOWS)]
    glo = [(int(off_lo[g[0]]), int(off_lo[g[-1] + 1])) for g in groups]
    ghi = [(int(off_hi[g[0]]), int(off_hi[g[-1] + 1])) for g in groups]
    cap_lo = max(b - a for a, b in glo) // 128
    cap_hi = max(b - a for a, b in ghi) // 128

    nc = bacc.Bacc("TRN2", target_bir_lowering=False, debug=False,
                   num_devices=N_CORES, num_swdge_queues=4)
    dt = mybir.dt

    # ---- kernel I/O -----------------------------------------------------
    t_xT = nc.dram_tensor("xT_own", [128, PER_CORE], dt.float32, kind="ExternalInput")
    t_W = [nc.dram_tensor(f"W{i+1}m", [F, F], dt.float32, kind="ExternalInput") for i in range(3)]
    t_b = [nc.dram_tensor(f"b{i+1}m", [128, F], dt.float32, kind="ExternalInput") for i in range(3)]
    t_dinv = nc.dram_tensor("dinv_own", [128, N_WIN], dt.float32, kind="ExternalInput")
    t_iota3 = nc.dram_tensor("iota3", [128, 128, NBMAX], dt.bfloat16, kind="ExternalInput")
    t_ilo = nc.dram_tensor("idx_lo", [128, n_lo // 16], dt.int16, kind="ExternalInput")
    t_ihi = nc.dram_tensor("idx_hi", [128, n_hi // 16], dt.int16, kind="ExternalInput")
    t_dlo = nc.dram_tensor("dl_lo", [128, n_lo // 128], dt.bfloat16, kind="ExternalInput")
    t_dhi = nc.dram_tensor("dl_hi", [128, n_hi // 128], dt.bfloat16, kind="ExternalInput")
    t_out = nc.dram_tensor("h_out", [PER_CORE, F_OUT], dt.float32, kind="ExternalOutput")

    with TileContext(nc) as tc:
        nc.gpsimd.load_library(library_config.mlp)
        with tc.tile_pool(name="const", bufs=1) as cpool, \
             tc.tile_pool(name="state", bufs=1) as spool, \
             tc.tile_pool(name="gath", bufs=2) as gpool, \
             tc.tile_pool(name="sbld", bufs=2) as sbld, \
             tc.tile_pool(name="work", bufs=3) as wpool, \
             tc.tile_pool(name="psA", bufs=2, space="PSUM") as psA, \
             tc.tile_pool(name="psB", bufs=2, space="PSUM") as psB, \
             tc.tile_pool(name="psT", bufs=2, space="PSUM") as psT, \
             tc.tile_pool(name="dram", bufs=1, space="DRAM") as dpool:

            # ---- constants ----
            c_W = [cpool.tile([F, F], dt.float32, tag=f"W{i}", name=f"cW{i}") for i in range(3)]
            c_b = [cpool.tile([128, F], dt.float32, tag=f"b{i}", name=f"cb{i}") for i in range(3)]
            c_dinv = cpool.tile([128, N_WIN], dt.float32, tag="dinv", name="dinv")
            c_iota3 = cpool.tile([128, 128, NBMAX], dt.bfloat16, tag="iota3", name="iota3")
            c_ilo = cpool.tile([128, n_lo // 16], dt.int16, tag="ilo", name="ilo")
            c_ihi = cpool.tile([128, n_hi // 16], dt.int16, tag="ihi", name="ihi")
            c_dlo = cpool.tile([128, n_lo // 128], dt.bfloat16, tag="dlo", name="dlo")
            c_dhi = cpool.tile([128, n_hi // 128], dt.bfloat16, tag="dhi", name="dhi")
            c_ident = cpool.tile([128, 128], dt.float32, tag="ident", name="ident")
            for i in range(3):
                nc.sync.dma_start(c_W[i][:], t_W[i][:])
                nc.sync.dma_start(c_b[i][:], t_b[i][:])
            nc.sync.dma_start(c_dinv[:], t_dinv[:])
            nc.sync.dma_start(c_iota3[:], t_iota3[:])
            nc.sync.dma_start(c_ilo[:], t_ilo[:])
            nc.sync.dma_start(c_ihi[:], t_ihi[:])
            nc.sync.dma_start(c_dlo[:], t_dlo[:])
            nc.sync.dma_start(c_dhi[:], t_dhi[:])
            make_identity(nc, c_ident[:])

            # ---- persistent state ----
            hT = [spool.tile([128, PER_CORE], dt.float32, tag="hT_a", name="hT_a"),
                  spool.tile([128, PER_CORE], dt.float32, tag="hT_b", name="hT_b")]
            nc.sync.dma_start(hT[0][:], t_xT[:])
            y_sb = spool.tile([128, N_WIN, F], dt.bfloat16, tag="y_sb", name="y_sb")
            out_sb = spool.tile([128, N_WIN, F_OUT], dt.float32, tag="out_sb", name="out_sb")

            y_As = [dpool.tile([A_ROWS, F], dt.bfloat16, addr_space="Shared",
                               name=f"y_A{i}") for i in range(3)]
            y_Bs = [dpool.tile([B_ROWS, F], dt.bfloat16, addr_space="Shared",
                               name=f"y_B{i}") for i in range(3)]
            ag_As = [dpool.tile([A_PC, F], dt.bfloat16, name=f"ag_A{i}")
                     for i in range(3)]
            ag_Bs = [dpool.tile([B_PC, F], dt.bfloat16, name=f"ag_B{i}")
                     for i in range(3)]

            def emit_y(layer, w):
                """y[w] = dinv * (h @ W[layer]) for layer's aggregation, plus
                the per-window push into the exchange staging buffer and the
                half-table AllGather when a half completes."""
                h_in = hT[layer % 2]
                ps = psA.tile([128, F], dt.float32, tag="psA", space="PSUM")
                nc.tensor.matmul(ps[:], lhsT=h_in[:, w * 128:(w + 1) * 128],
                                 rhs=c_W[layer][:], start=True, stop=True)
                nc.vector.tensor_scalar(
                    out=y_sb[:, w, :], in0=ps[:],
                    scalar1=c_dinv[:, w:w + 1], scalar2=None,
                    op0=mybir.AluOpType.mult)
                if w < A_WINS:
                    agv = ag_As[layer][:].rearrange("(t p) f -> p t f", p=128)
                    nc.sync.dma_start(agv[:, w:w + 1, :], y_sb[:, w:w + 1, :])
                else:
                    agv = ag_Bs[layer][:].rearrange("(t p) f -> p t f", p=128)
                    nc.sync.dma_start(agv[:, w - A_WINS:w - A_WINS + 1, :],
                                      y_sb[:, w:w + 1, :])
                if w == A_WINS - 1:
                    nc.gpsimd.collective_compute(
                        "AllGather", mybir.AluOpType.bypass,
                        replica_groups=[list(range(N_CORES))],
                        ins=[ag_As[layer].opt()], outs=[y_As[layer].opt()])
                elif w == N_WIN - 1:
                    nc.gpsimd.collective_compute(
                        "AllGather", mybir.AluOpType.bypass,
                        replica_groups=[list(range(N_CORES))],
                        ins=[ag_Bs[layer].opt()], outs=[y_Bs[layer].opt()])

            def build_S(pool_tag, dl_tile, B0, nb):
                """One-hot scatter blocks for a (window, stream):
                S[e, d, j] = (dl[e, B0+j] == d), bf16."""
                S = sbld.tile([128, 128, NBMAX], dt.bfloat16, tag=pool_tag,
                              name=pool_tag)
                dl_b = dl_tile[:, B0:B0 + nb].unsqueeze(1).broadcast_to(
                    [128, 128, nb])
                nc.vector.tensor_tensor(
                    out=S[:, :, :nb], in0=dl_b, in1=c_iota3[:, :, :nb],
                    op=mybir.AluOpType.is_equal)
                return S

            # ---- layer 0 phase A (prologue) ----
            for t in range(N_WIN):
                emit_y(0, t)

            for layer in range(3):
                h_out = hT[(layer + 1) % 2]
                y_A, y_B = y_As[layer], y_Bs[layer]
                # ---- phase B: gather + one-hot matmul aggregation ----
                for gi, g in enumerate(groups):
                    lo_a, lo_b = glo[gi]
                    hi_a, hi_b = ghi[gi]
                    m_lo = gpool.tile([128, cap_lo, F], dt.bfloat16, tag="mlo", name="mlo")
                    m_hi = gpool.tile([128, cap_hi, F], dt.bfloat16, tag="mhi", name="mhi")
                    # split each stream's block range in half -> 4 queues
                    for st, (a, b, m_t, src_t, idx_t, q0) in enumerate((
                            (lo_a, lo_b, m_lo, y_A, c_ilo, 0),
                            (hi_a, hi_b, m_hi, y_B, c_ihi, 2))):
                        nblk_st = (b - a) // 128
                        cut = a + (nblk_st // 2) * 128
                        for (aa, bb), q in (((a, cut), q0), ((cut, b), q0 + 1)):
                            if bb > aa:
                                nc.gpsimd.dma_gather(
                                    out_ap=m_t[:, (aa - a) // 128:(bb - a) // 128, :],
                                    in_ap=src_t[:],
                                    idxs_ap=idx_t[:, aa // 16:bb // 16],
                                    num_idxs=bb - aa, num_idxs_reg=bb - aa,
                                    elem_size=F, queue_num=q,
                                    single_packet=False)
                    for w in g:
                        nb_lo = int(blk_lo[w])
                        nb_hi = int(blk_hi[w])
                        nblk = nb_lo + nb_hi
                        B_lo = int(off_lo[w]) // 128
                        B_hi = int(off_hi[w]) // 128
                        S_lo = build_S("Slo", c_dlo, B_lo, nb_lo)
                        S_hi = build_S("Shi", c_dhi, B_hi, nb_hi)
                        agg = psB.tile([128, F], dt.float32, tag="agg", space="PSUM")
                        k = 0
                        for j in range(nb_lo):
                            nc.tensor.matmul(
                                agg[:], lhsT=S_lo[:, :, j],
                                rhs=m_lo[:, B_lo - lo_a // 128 + j, :],
                                start=(k == 0), stop=(k == nblk - 1))
                            k += 1
                        for j in range(nb_hi):
                            nc.tensor.matmul(
                                agg[:], lhsT=S_hi[:, :, j],
                                rhs=m_hi[:, B_hi - hi_a // 128 + j, :],
                                start=(k == 0), stop=(k == nblk - 1))
                            k += 1
                        # ---- epilogue: h = dinv*agg + b ----
                        hb = wpool.tile([128, F], dt.float32, tag="hb", name="hb")
                        nc.vector.scalar_tensor_tensor(
                            out=hb[:], in0=agg[:],
                            scalar=c_dinv[:, w:w + 1], in1=c_b[layer][:],
                            op0=mybir.AluOpType.mult, op1=mybir.AluOpType.add)
                        if layer < 2:
                            tp = psT.tile([128, 128], dt.float32, tag="tp", space="PSUM")
                            nc.tensor.transpose(tp[:], hb[:], c_ident[:])
                            nc.vector.tensor_copy(
                                out=h_out[:, w * 128:(w + 1) * 128], in_=tp[:])
                            # phase-A fold: y for layer+1 from the fresh h
                            emit_y(layer + 1, w)
                        else:
                            nc.vector.tensor_scalar(
                                out=out_sb[:, w, :], in0=hb[:, :F_OUT],
                                scalar1=0.0, scalar2=None,
                                op0=mybir.AluOpType.max)
            nc.sync.dma_start(
                t_out[:].rearrange("(t p) f -> p t f", p=128), out_sb[:])

    nc.compile()

    # ---- per-core inputs ----
    xT_all = np.zeros((128, N_PAD), np.float32)
    xT_all[:, :N_NODES] = np.asarray(x, np.float32).T
    iota3 = np.broadcast_to(
        np.arange(128, dtype=np.float32)[None, :, None],
        (128, 128, NBMAX)).astype(BF16)
    in_maps = []
    for c in range(N_CORES):
        rows = slice(c * PER_CORE, (c + 1) * PER_CORE)
        din = dinv_pad[rows].reshape(N_WIN, 128).T.copy()  # [128, N_WIN]
        in_map = {
            "xT_own": np.ascontiguousarray(xT_all[:, rows]),
            "dinv_own": din,
            "iota3": iota3.copy(),
            "idx_lo": _wrap_idx16(idx_lo[c]),
            "idx_hi": _wrap_idx16(idx_hi[c]),
            "dl_lo": dl_lo[c].reshape(-1, 128).T.astype(BF16).copy(),
            "dl_hi": dl_hi[c].reshape(-1, 128).T.astype(BF16).copy(),
        }
        for i in range(3):
            in_map[f"W{i+1}m"] = Ws[i].copy()
            in_map[f"b{i+1}m"] = np.broadcast_to(bs[i], (128, F)).copy()
        in_maps.append(in_map)

    if sim:
        from concourse.bass_interp import MultiCoreSim
        mcs = MultiCoreSim(nc, num_cores=N_CORES, trace=False,
                           require_finite=False, require_nnan=False)
        for ci, core in enumerate(mcs.cores.values()):
            for k, v in in_maps[ci].items():
                core.tensor(k)[:] = v
        mcs.simulate(check_with_hw=False)
        outs = [np.asarray(core.tensor("h_out"))
                for core in mcs.cores.values()]
        res = None
    else:
        res = bass_utils.run_bass_kernel_spmd(
            nc, in_maps, core_ids=list(range(N_CORES)), trace=trace)
        outs = [r["h_out"] for r in res.results]
    full = np.concatenate(outs, axis=0)[:N_NODES]
    return full, res


def kernel(**inputs) -> np.ndarray:
    edge_index = np.asarray(inputs["edge_index"])
    prep = _preprocess(edge_index)
    out, _ = _build_and_run(inputs, *prep)
    return out
